# revision 26
# baseline (speedup 1.0000x reference)
"""Trainium2 Bass kernel for batched multi-head attention.

Problem: query/key/value [B=2, H=16, S=2048, D=64] fp32, per-(b,h) divisor
`inv_scale_factor` [B, H, 1, 1].  out = softmax(Q K^T / inv_scale) V.

Sharding: the 32 (b,h) heads are split across 8 NeuronCores, 4 heads per
core, fully data-parallel (no collectives).

Per-core design (v2 — ACT-stream-first):
  - The saturated engine is ACT (exp over all S*S scores).  Everything else
    is scheduled around an uninterrupted EXP stream with as few, as large
    ACTIVATEs as possible (cost ~(N+352)/1.2 ns each).
  - Q^T / K^T are produced by the DMA xbar transpose engine (fp16,
    per-128-column-block transposes executed inline on the SP queue
    engine), not the PE: natural fp32 load -> DVE cast into zero-padded
    [kv,128] blocks -> xbar calls.  This frees the PE and the two PSUM
    banks the old identity-matmul transposes used.
  - Engine split for the serial queue engines: bulk loads are issued from
    the Activation HWDGE queue (they carry no input waits, so they can
    never stall EXP issue for long), while the SP queue runs the xbar
    transposes + stores (their inputs come from the DVE, which is nearly
    idle).
  - PSUM (8 banks): big score tile [128,2048] (4), small [128,1024] (2),
    PV accumulator [65,512] double-buffered (2).  Per q-block (512 q rows)
    the 16 kv tiles are processed as chunks [B4,S2,B4,S2,B2,S2] which
    strictly alternates the big/small pools, including across q-block and
    head wraps.
  - Software pipeline per chunk i: QK(i+1) is emitted BEFORE PV(i-1) so
    the in-order PE starts the next chunk's scores the moment the current
    EXP retires; PV trails by one chunk (pt pool depth covers it).
  - PV uses V augmented with a ones column ([kv, 65] fp16 stationary), so
    the softmax denominator falls out of the same accumulating matmul.
  - Epilogue: [65,512] accumulator -> fp16 -> one xbar call gives the
    [q,d]-major layout, then per-q-tile reciprocal + scale on DVE, DMA out.
    No PE work in staging or epilogue.
"""

import numpy as np

import concourse.bass as bass
import concourse.tile as tile
from concourse import bacc, mybir
from concourse.bass_utils import run_bass_kernel_spmd
from concourse.masks import make_identity

F32 = mybir.dt.float32
F16 = mybir.dt.float16
EXP = mybir.ActivationFunctionType.Exp
LNP = float(np.log(128.0))

B, H, SQ, SKV, D = 2, 16, 2048, 2048, 64
N_CORES = 8
HEADS_PER_CORE = (B * H) // N_CORES  # 4

QB = 512                  # q rows per accumulation block
NQB = SQ // QB            # 4 q-blocks per head
NKV = SKV // 128          # 16 kv tiles per head
NTQ = SQ // 128           # 16 q tiles per head
CHUNKS = [("B", 0, 4), ("S", 4, 2), ("B", 6, 2), ("S", 8, 2),
          ("B", 10, 4), ("S", 14, 2)]
CPQB = len(CHUNKS)        # chunks per q-block
CPH = CPQB * NQB          # chunks per head


def build_attention(nh=HEADS_PER_CORE, num_devices=N_CORES,
                    enable_asserts=False):
    nc = bacc.Bacc("TRN2", target_bir_lowering=False, debug=False,
                   enable_asserts=enable_asserts, num_devices=num_devices)

    q_dram = nc.dram_tensor("query", [nh, SQ, D], F32, kind="ExternalInput").ap()
    k_dram = nc.dram_tensor("key", [nh, SKV, D], F32, kind="ExternalInput").ap()
    v_dram = nc.dram_tensor("value", [nh, SKV, D], F32, kind="ExternalInput").ap()
    inv_dram = nc.dram_tensor("inv_scale", [1, nh], F32, kind="ExternalInput").ap()
    o_dram = nc.dram_tensor("out", [nh, SQ, D], F32, kind="ExternalOutput").ap()

    with tile.TileContext(nc) as tc:
        _attention_body(tc, o_dram, q_dram, k_dram, v_dram, inv_dram, nh)

    nc.compile()
    return nc


def _attention_body(tc, o_dram, q_dram, k_dram, v_dram, inv_dram, nh):
    nc = tc.nc
    from contextlib import ExitStack
    with ExitStack() as ctx:
        const = ctx.enter_context(tc.tile_pool(name="const", bufs=1))
        knatp = ctx.enter_context(tc.tile_pool(name="knat", bufs=2))
        qnatp = ctx.enter_context(tc.tile_pool(name="qnat", bufs=2))
        vnatp = ctx.enter_context(tc.tile_pool(name="vnat", bufs=2))
        khpp = ctx.enter_context(tc.tile_pool(name="khp", bufs=2))
        qhpp = ctx.enter_context(tc.tile_pool(name="qhp", bufs=2))
        ktp = ctx.enter_context(tc.tile_pool(name="kt", bufs=2))
        qtp = ctx.enter_context(tc.tile_pool(name="qt", bufs=2))
        vaugp = ctx.enter_context(tc.tile_pool(name="vaug", bufs=2))
        ptp = ctx.enter_context(tc.tile_pool(name="pt", bufs=6))
        osbp = ctx.enter_context(tc.tile_pool(name="osb", bufs=2))
        trp = ctx.enter_context(tc.tile_pool(name="tr", bufs=2))
        finp = ctx.enter_context(tc.tile_pool(name="fin", bufs=4))
        recp = ctx.enter_context(tc.tile_pool(name="rec", bufs=8))
        bigp = ctx.enter_context(tc.tile_pool(name="bigps", bufs=1, space="PSUM"))
        smallp = ctx.enter_context(tc.tile_pool(name="smps", bufs=1, space="PSUM"))
        outp = ctx.enter_context(tc.tile_pool(name="outps", bufs=2, space="PSUM"))

        # ---- constants ----
        ones_row = const.tile([1, 128], F32)
        nc.vector.memset(ones_row[:], 1.0)
        bias_col = const.tile([128, 1], F32)
        nc.vector.memset(bias_col[:], -LNP)
        ident = const.tile([128, 128], F32)
        make_identity(nc, ident[:])
        ident_h = const.tile([128, 128], F16)
        nc.vector.tensor_copy(ident_h[:], ident[:])
        # warm the ACT exp table set before any real dependency exists
        warm = const.tile([1, 1], F32)
        nc.scalar.activation(warm[:], ones_row[0:1, 0:1], EXP,
                             bias=bias_col[0:1, :])
        inv_sb = const.tile([1, nh], F32)
        nc.sync.dma_start(inv_sb[:], inv_dram[:])
        recip_sb = const.tile([1, nh], F32)
        nc.vector.reciprocal(recip_sb[:], inv_sb[:])
        # broadcast 1/inv_scale across partitions via a tiny K=1 matmul
        bps = smallp.tile([128, 1024], F32, tag="sc", name="bcast")
        nc.tensor.matmul(bps[0:128, 0:nh], ones_row[0:1, 0:128],
                         recip_sb[0:1, 0:nh], start=True, stop=True)
        scale_all = const.tile([128, nh], F32)
        nc.vector.tensor_copy(scale_all[:], bps[0:128, 0:nh])

        # ---- per-head staging, split so loads can run 2 heads ahead ----
        nat_by_head = {}
        tiles_by_head = {}

        def stage_loads(h):
            """Allocate the natural-layout fp32 tiles for head h and
            return quarter-granular load closures (quarters bound the
            per-ring serialization latency to ~8us)."""
            kdr = k_dram[h].rearrange("(t p) e -> p t e", p=128)
            qdr = q_dram[h].rearrange("(t p) e -> p t e", p=128)
            vdr = v_dram[h].rearrange("(t p) e -> p t e", p=128)
            knat = knatp.tile([128, NKV * D], F32, tag="knat", name="knat")
            qnat = qnatp.tile([128, NTQ * D], F32, tag="qnat", name="qnat")
            vnat = vnatp.tile([128, NKV * (D + 1)], F32, tag="vnat", name="vnat")
            nat_by_head[h] = (knat, qnat, vnat)
            knv = knat[:].rearrange("p (t e) -> p t e", e=D)
            qnv = qnat[:].rearrange("p (t e) -> p t e", e=D)
            vnv = vnat[:].rearrange("p (t e) -> p t e", e=D + 1)

            def l_k(t0, t1):
                nc.sync.dma_start(knv[:, t0:t1, :], kdr[:, t0:t1, :])

            def l_q(t0, t1):
                nc.sync.dma_start(qnv[:, t0:t1, :], qdr[:, t0:t1, :])

            def m_v():
                nc.gpsimd.memset(vnat[:], 1.0)

            def l_v(t0, t1):
                nc.sync.dma_start(vnv[:, t0:t1, 0:D], vdr[:, t0:t1, :])

            loads = {"l_k": l_k, "l_q": l_q, "l_v": l_v, "m_v": m_v}
            cl = [(0, m_v), (0, lambda: l_k(0, 8)),
                  (1, lambda: l_k(8, 16)), (2, lambda: l_q(0, 8)),
                  (3, lambda: l_q(8, 16)), (4, lambda: l_v(0, 8)),
                  (5, lambda: l_v(8, 16))]
            return loads, cl

        def stage_xforms(h):
            """fp16 casts into zero-padded 128-col blocks + xbar block
            transposes.  Scheduled late enough that the loads (2 heads of
            lead time) are surely complete, so the in-order SP queue never
            blocks on the xbars' inputs."""
            knat, qnat, vnat = nat_by_head[h]
            khp = khpp.tile([128, NKV * 128], F16, tag="khp", name="khp")
            qhp = qhpp.tile([128, NTQ * 128], F16, tag="qhp", name="qhp")
            kt = ktp.tile([128, NKV * 128], F16, tag="kt", name="kt")
            qt = qtp.tile([128, NTQ * 128], F16, tag="qt", name="qt")
            vaug = vaugp.tile([128, NKV * (D + 1)], F16, tag="vaug", name="vaug")
            tiles_by_head[h] = (kt, qt, vaug)

            knv = knat[:].rearrange("p (t e) -> p t e", e=D)
            qnv = qnat[:].rearrange("p (t e) -> p t e", e=D)
            vnv = vnat[:].rearrange("p (t e) -> p t e", e=D + 1)
            khv = khp[:].rearrange("p (t e) -> p t e", e=128)
            qhv = qhp[:].rearrange("p (t e) -> p t e", e=128)
            ktv = kt[:].rearrange("p (t e) -> p t e", e=128)
            qtv = qt[:].rearrange("p (t e) -> p t e", e=128)
            vv = vaug[:].rearrange("p (t e) -> p t e", e=D + 1)
            sh = scale_all[:, h:h + 1]

            def m_kz():
                nc.vector.memset(khv[:, :, D:128], 0.0)

            def m_qz():
                nc.vector.memset(qhv[:, :, D:128], 0.0)

            def c_k(t0, t1):
                nc.vector.tensor_copy(khv[:, t0:t1, 0:D], knv[:, t0:t1, :])

            def c_q(t0, t1):
                nc.vector.tensor_scalar_mul(qhv[:, t0:t1, 0:D],
                                            qnv[:, t0:t1, :], sh)

            def c_v(t0, t1):
                nc.vector.tensor_copy(vv[:, t0:t1, :], vnv[:, t0:t1, :])

            def x_k(t0, t1):
                nc.sync.dma_start(ktv[:, t0:t1, :],
                                  khp[:, t0 * 128:t1 * 128], transpose=True)

            def x_q(t0, t1):
                nc.sync.dma_start(qtv[:, t0:t1, :],
                                  qhp[:, t0 * 128:t1 * 128], transpose=True)

            xf = {"m_kz": m_kz, "m_qz": m_qz, "c_k": c_k, "c_q": c_q,
                  "c_v": c_v, "x_k": x_k, "x_q": x_q}
            # head 1's loads only start at chunk ~2, so its transforms sit
            # later to avoid in-line queue-engine waits; later heads' loads
            # ran two heads ago and their transforms can spread out early.
            b = 16 if h == 1 else 10
            cl = []
            if h < 2:
                cl += [(b - 2, m_kz), (b - 1, m_qz)]
            cl += [(b + 0, lambda: c_k(0, 8)), (b + 1, lambda: x_k(0, 8)),
                   (b + 1, lambda: c_q(0, 8)), (b + 2, lambda: x_q(0, 8)),
                   (b + 2, lambda: c_v(0, 8)),
                   (b + 3, lambda: c_k(8, 16)), (b + 4, lambda: x_k(8, 16)),
                   (b + 4, lambda: c_q(8, 16)), (b + 5, lambda: x_q(8, 16)),
                   (b + 5, lambda: c_v(8, 16))]
            return xf, cl

        def stage_head0_prefix():
            """Head 0 prologue, emitted inline.  Ordered so the SP queue
            issues the critical K/Q tile loads first, weaves the xbars in
            as their cast inputs complete, and defers V (the trailing PV
            tolerates ~5 chunks of lag via the pt pool depth)."""
            loads, _ = stage_loads(0)
            l_k, l_q, l_v, m_v = (loads["l_k"], loads["l_q"],
                                  loads["l_v"], loads["m_v"])
            xf, _ = stage_xforms(0)
            m_kz, m_qz = xf["m_kz"], xf["m_qz"]
            c_k, c_q, c_v = xf["c_k"], xf["c_q"], xf["c_v"]
            x_k, x_q = xf["x_k"], xf["x_q"]
            prefix = [
                m_kz, m_qz, m_v,
                lambda: l_k(0, 2), lambda: l_k(2, 4),
                lambda: l_q(0, 2), lambda: l_q(2, 4),
                lambda: l_k(4, 8), lambda: l_q(4, 8),
                lambda: l_k(8, 12), lambda: l_v(0, 4),
                lambda: c_k(0, 4), lambda: c_q(0, 4),
                lambda: x_k(0, 4), lambda: x_q(0, 4),
                lambda: l_k(12, 16), lambda: l_v(4, 8),
                lambda: c_k(4, 8), lambda: x_k(4, 8),
                lambda: l_q(8, 12), lambda: l_v(8, 12),
                lambda: c_q(4, 8), lambda: x_q(4, 8),
                lambda: l_q(12, 16), lambda: l_v(12, 16),
                lambda: c_k(8, 12), lambda: x_k(8, 12),
                lambda: c_k(12, 16), lambda: x_k(12, 16),
                lambda: c_v(0, 4), lambda: c_v(4, 8),
                lambda: c_v(8, 12), lambda: c_v(12, 16),
            ]
            for f in prefix:
                f()
            rest = [(6, lambda: c_q(8, 16)), (7, lambda: x_q(8, 16))]
            return rest

        # ---- flat chunk schedule across all heads ----
        sched = []
        for h in range(nh):
            for qb in range(NQB):
                for ci, (kind, t0, nt) in enumerate(CHUNKS):
                    sched.append((h, qb, kind, t0, nt, ci == CPQB - 1))
        nchunks = len(sched)

        stage_q = []   # (absolute_chunk_pos, closure)
        epi_q = []

        stage_q.extend(stage_head0_prefix())
        if nh > 1:
            _, cl = stage_loads(1)
            stage_q.extend((2 + p, f) for p, f in cl)

        osb_count = [0]

        def emit_qk(i):
            h, qb, kind, t0, nt, _ = sched[i]
            kt, qt, _ = tiles_by_head[h]
            pool = bigp if kind == "B" else smallp
            sc = pool.tile([128, nt * QB], F32, tag="sc", name=f"sc{kind}")
            for j in range(nt):
                t = t0 + j
                nc.tensor.matmul(sc[:, j * QB:(j + 1) * QB],
                                 kt[:, t * 128:(t + 1) * 128],
                                 qt[:, qb * QB:(qb + 1) * QB],
                                 start=True, stop=True)
            return sc

        out_ps_by_qb = {}

        def emit_pv(i):
            h, qb, kind, t0, nt, qb_last = sched[i]
            _, _, vaug = tiles_by_head[h]
            if t0 == 0:
                out_ps_by_qb[(h, qb)] = outp.tile([65, QB], F32, tag="out",
                                                  name="out_ps")
            out_ps = out_ps_by_qb[(h, qb)]
            pt = pt_by_chunk.pop(i)
            for j in range(nt):
                t = t0 + j
                nc.tensor.matmul(out_ps[0:65, :],
                                 vaug[:, t * 65:(t + 1) * 65],
                                 pt[:, j * QB:(j + 1) * QB],
                                 start=(t == 0), stop=(t == NKV - 1))
            if qb_last:
                eps = make_epilogue(h, qb, out_ps)
                epi_q.insert(0, eps[0])   # free the accumulator slot first
                epi_q.extend(eps[1:])

        def make_epilogue(h, qb, out_ps):
            cell = {}

            def c_copy():
                osb = osbp.tile([80, QB], F16, tag="osb", name="osb")
                if osb_count[0] < 2:
                    nc.vector.memset(osb[64:80, :], 0.0)
                osb_count[0] += 1
                nc.vector.tensor_copy(osb[0:65, :], out_ps[0:65, :])
                cell["osb"] = osb

            def c_tr():
                tr = trp.tile([128, 4 * 80], F16, tag="tr", name="tr")
                nc.sync.dma_start(
                    tr[:].rearrange("p (b c) -> p b c", c=80),
                    cell["osb"][:], transpose=True)
                fin = finp.tile([128, 4 * D], F32, tag="fin", name="fin")
                cell["tr"], cell["fin"] = tr, fin

            def c_fin(j):
                tr, fin = cell["tr"], cell["fin"]
                rec = recp.tile([128, 1], F32, tag="rec", name="rec")
                nc.vector.reciprocal(rec[:],
                                     tr[:, j * 80 + D:j * 80 + D + 1])
                nc.vector.tensor_scalar_mul(
                    fin[:, j * D:(j + 1) * D], tr[:, j * 80:j * 80 + D],
                    rec[:])

            odr = o_dram[h].rearrange("(t p) e -> p t e", p=128)

            def fview():
                return cell["fin"][:].rearrange("p (t e) -> p t e", e=D)

            if h == nh - 1 and qb == NQB - 1:
                dmas = [lambda j=j: nc.sync.dma_start(
                    odr[:, qb * 4 + j:qb * 4 + j + 1, :],
                    fview()[:, j:j + 1, :]) for j in range(4)]
            else:
                dmas = [lambda: nc.sync.dma_start(
                    odr[:, qb * 4:(qb + 1) * 4, :], fview())]

            return [c_copy, c_tr] + \
                [lambda j=j: c_fin(j) for j in range(4)] + dmas

        # ---- main pipelined loop over chunks ----
        # per iteration i: QK(i+1) -> EXP(i) -> PV(i-1); the in-order PE
        # then runs QK(i+1) the instant EXP(i-?)'s PSUM read retires,
        # instead of burning that window on PV.
        pt_by_chunk = {}
        sc_cur = emit_qk(0)
        for i in range(nchunks):
            h, qb, kind, t0, nt, qb_last = sched[i]
            # staging plan: loads 2 heads ahead, transforms 1 head ahead
            # (head 0/1 bootstrap compressed around the prologue).
            if i == 4 and nh > 1:
                _, cl = stage_xforms(1)
                stage_q.extend((i + p, f) for p, f in cl)
            if i == 8 and nh > 2:
                _, cl = stage_loads(2)
                stage_q.extend((i + p, f) for p, f in cl)
            if i == CPH and nh > 2:
                if nh > 3:
                    _, cl = stage_loads(3)
                    stage_q.extend((i + p, f) for p, f in cl)
                _, cl = stage_xforms(2)
                stage_q.extend((i + p, f) for p, f in cl)
            if i == 2 * CPH and nh > 3:
                _, cl = stage_xforms(3)
                stage_q.extend((i + p, f) for p, f in cl)
            sc_next = emit_qk(i + 1) if i + 1 < nchunks else None
            pt = ptp.tile([128, nt * QB], F16, tag="pt", name="pt")
            nc.scalar.activation(pt[:], sc_cur[:], EXP, bias=bias_col[:],
                                 scale=1.0)
            pt_by_chunk[i] = pt
            if i > 0:
                emit_pv(i - 1)
            sc_cur = sc_next
            # drain scheduled staging work, then epilogue pieces
            due = [f for p, f in stage_q if p <= i]
            stage_q = [(p, f) for p, f in stage_q if p > i]
            for f in due:
                f()
            ne = 0
            while ne < 2 and epi_q:
                epi_q.pop(0)()
                ne += 1
        emit_pv(nchunks - 1)
        while stage_q:
            stage_q.pop(0)[1]()
        while epi_q:
            epi_q.pop(0)()


_NC_CACHE = {}


def _get_program():
    key = "full"
    if key not in _NC_CACHE:
        _NC_CACHE[key] = build_attention()
    return _NC_CACHE[key]


def kernel(query, key, value, inv_scale_factor):
    """Full-input entry point: shard over 8 cores, run, gather."""
    nc = _get_program()
    q = np.ascontiguousarray(query, dtype=np.float32).reshape(B * H, SQ, D)
    k = np.ascontiguousarray(key, dtype=np.float32).reshape(B * H, SKV, D)
    v = np.ascontiguousarray(value, dtype=np.float32).reshape(B * H, SKV, D)
    inv = np.ascontiguousarray(inv_scale_factor, dtype=np.float32).reshape(B * H)

    hpc = HEADS_PER_CORE
    in_maps = []
    for c in range(N_CORES):
        s = slice(c * hpc, (c + 1) * hpc)
        in_maps.append({
            "query": q[s],
            "key": k[s],
            "value": v[s],
            "inv_scale": inv[s].reshape(1, hpc),
        })
    res = run_bass_kernel_spmd(nc, in_maps, core_ids=list(range(N_CORES)))
    out = np.concatenate([res.results[c]["out"] for c in range(N_CORES)], axis=0)
    return out.reshape(B, H, SQ, D)


# revision 32
# speedup vs baseline: 1.0017x; 1.0017x over previous
"""Trainium2 Bass kernel for batched multi-head attention.

Problem: query/key/value [B=2, H=16, S=2048, D=64] fp32, per-(b,h) divisor
`inv_scale_factor` [B, H, 1, 1].  out = softmax(Q K^T / inv_scale) V.

Sharding: the 32 (b,h) heads are split across 8 NeuronCores, 4 heads per
core, fully data-parallel (no collectives).

Per-core design (v2 — ACT-stream-first):
  - The saturated engine is ACT (exp over all S*S scores).  Everything else
    is scheduled around an uninterrupted EXP stream with as few, as large
    ACTIVATEs as possible (cost ~(N+352)/1.2 ns each).
  - Q^T / K^T are produced by the DMA xbar transpose engine (fp16,
    per-128-column-block transposes executed inline on the SP queue
    engine), not the PE: natural fp32 load -> DVE cast into zero-padded
    [kv,128] blocks -> xbar calls.  This frees the PE and the two PSUM
    banks the old identity-matmul transposes used.
  - Engine split for the serial queue engines: bulk loads are issued from
    the Activation HWDGE queue (they carry no input waits, so they can
    never stall EXP issue for long), while the SP queue runs the xbar
    transposes + stores (their inputs come from the DVE, which is nearly
    idle).
  - PSUM (8 banks): big score tile [128,2048] (4), small [128,1024] (2),
    PV accumulator [65,512] double-buffered (2).  Per q-block (512 q rows)
    the 16 kv tiles are processed as chunks [B4,S2,B4,S2,B2,S2] which
    strictly alternates the big/small pools, including across q-block and
    head wraps.
  - Software pipeline per chunk i: QK(i+1) is emitted BEFORE PV(i-1) so
    the in-order PE starts the next chunk's scores the moment the current
    EXP retires; PV trails by one chunk (pt pool depth covers it).
  - PV uses V augmented with a ones column ([kv, 65] fp16 stationary), so
    the softmax denominator falls out of the same accumulating matmul.
  - Epilogue: [65,512] accumulator -> fp16 -> one xbar call gives the
    [q,d]-major layout, then per-q-tile reciprocal + scale on DVE, DMA out.
    No PE work in staging or epilogue.
"""

import numpy as np

import concourse.bass as bass
import concourse.tile as tile
from concourse import bacc, mybir
from concourse.bass_utils import run_bass_kernel_spmd
from concourse.masks import make_identity

F32 = mybir.dt.float32
F16 = mybir.dt.float16
EXP = mybir.ActivationFunctionType.Exp
LNP = float(np.log(128.0))

B, H, SQ, SKV, D = 2, 16, 2048, 2048, 64
N_CORES = 8
HEADS_PER_CORE = (B * H) // N_CORES  # 4

QB = 512                  # q rows per accumulation block
NQB = SQ // QB            # 4 q-blocks per head
NKV = SKV // 128          # 16 kv tiles per head
NTQ = SQ // 128           # 16 q tiles per head
CHUNKS = [("B", 0, 4), ("S", 4, 2), ("B", 6, 4), ("S", 10, 2),
          ("B", 12, 2), ("S", 14, 2)]
CPQB = len(CHUNKS)        # chunks per q-block
CPH = CPQB * NQB          # chunks per head


def build_attention(nh=HEADS_PER_CORE, num_devices=N_CORES,
                    enable_asserts=False):
    nc = bacc.Bacc("TRN2", target_bir_lowering=False, debug=False,
                   enable_asserts=enable_asserts, num_devices=num_devices)

    q_dram = nc.dram_tensor("query", [nh, SQ, D], F32, kind="ExternalInput").ap()
    k_dram = nc.dram_tensor("key", [nh, SKV, D], F32, kind="ExternalInput").ap()
    v_dram = nc.dram_tensor("value", [nh, SKV, D], F32, kind="ExternalInput").ap()
    inv_dram = nc.dram_tensor("inv_scale", [1, nh], F32, kind="ExternalInput").ap()
    o_dram = nc.dram_tensor("out", [nh, SQ, D], F32, kind="ExternalOutput").ap()

    with tile.TileContext(nc) as tc:
        _attention_body(tc, o_dram, q_dram, k_dram, v_dram, inv_dram, nh)

    nc.compile()
    return nc


def _attention_body(tc, o_dram, q_dram, k_dram, v_dram, inv_dram, nh):
    nc = tc.nc
    from contextlib import ExitStack
    with ExitStack() as ctx:
        const = ctx.enter_context(tc.tile_pool(name="const", bufs=1))
        knatp = ctx.enter_context(tc.tile_pool(name="knat", bufs=2))
        qnatp = ctx.enter_context(tc.tile_pool(name="qnat", bufs=2))
        vnatp = ctx.enter_context(tc.tile_pool(name="vnat", bufs=2))
        khpp = ctx.enter_context(tc.tile_pool(name="khp", bufs=2))
        qhpp = ctx.enter_context(tc.tile_pool(name="qhp", bufs=2))
        ktp = ctx.enter_context(tc.tile_pool(name="kt", bufs=2))
        qtp = ctx.enter_context(tc.tile_pool(name="qt", bufs=2))
        vaugp = ctx.enter_context(tc.tile_pool(name="vaug", bufs=2))
        ptp = ctx.enter_context(tc.tile_pool(name="pt", bufs=6))
        osbp = ctx.enter_context(tc.tile_pool(name="osb", bufs=2))
        trp = ctx.enter_context(tc.tile_pool(name="tr", bufs=2))
        finp = ctx.enter_context(tc.tile_pool(name="fin", bufs=4))
        recp = ctx.enter_context(tc.tile_pool(name="rec", bufs=8))
        bigp = ctx.enter_context(tc.tile_pool(name="bigps", bufs=1, space="PSUM"))
        smallp = ctx.enter_context(tc.tile_pool(name="smps", bufs=1, space="PSUM"))
        outp = ctx.enter_context(tc.tile_pool(name="outps", bufs=2, space="PSUM"))

        # ---- constants ----
        ones_row = const.tile([1, 128], F32)
        nc.vector.memset(ones_row[:], 1.0)
        bias_col = const.tile([128, 1], F32)
        nc.vector.memset(bias_col[:], -LNP)
        # warm the ACT exp table set before any real dependency exists
        warm = const.tile([1, 1], F32)
        nc.scalar.activation(warm[:], ones_row[0:1, 0:1], EXP,
                             bias=bias_col[0:1, :])
        inv_sb = const.tile([1, nh], F32)
        nc.sync.dma_start(inv_sb[:], inv_dram[:])
        recip_sb = const.tile([1, nh], F32)
        nc.vector.reciprocal(recip_sb[:], inv_sb[:])
        # broadcast 1/inv_scale across partitions via a tiny K=1 matmul
        bps = smallp.tile([128, 1024], F32, tag="sc", name="bcast")
        nc.tensor.matmul(bps[0:128, 0:nh], ones_row[0:1, 0:128],
                         recip_sb[0:1, 0:nh], start=True, stop=True)
        scale_all = const.tile([128, nh], F32)
        nc.vector.tensor_copy(scale_all[:], bps[0:128, 0:nh])

        # ---- per-head staging, split so loads can run 2 heads ahead ----
        nat_by_head = {}
        tiles_by_head = {}

        def stage_loads(h):
            """Allocate the natural-layout fp32 tiles for head h and
            return quarter-granular load closures (quarters bound the
            per-ring serialization latency to ~8us)."""
            kdr = k_dram[h].rearrange("(t p) e -> p t e", p=128)
            qdr = q_dram[h].rearrange("(t p) e -> p t e", p=128)
            vdr = v_dram[h].rearrange("(t p) e -> p t e", p=128)
            knat = knatp.tile([128, NKV * D], F32, tag="knat", name="knat")
            qnat = qnatp.tile([128, NTQ * D], F32, tag="qnat", name="qnat")
            vnat = vnatp.tile([128, NKV * (D + 1)], F32, tag="vnat", name="vnat")
            nat_by_head[h] = (knat, qnat, vnat)
            knv = knat[:].rearrange("p (t e) -> p t e", e=D)
            qnv = qnat[:].rearrange("p (t e) -> p t e", e=D)
            vnv = vnat[:].rearrange("p (t e) -> p t e", e=D + 1)

            def l_k(t0, t1):
                nc.sync.dma_start(knv[:, t0:t1, :], kdr[:, t0:t1, :])

            def l_q(t0, t1):
                nc.sync.dma_start(qnv[:, t0:t1, :], qdr[:, t0:t1, :])

            def m_v():
                nc.gpsimd.memset(vnat[:], 1.0)

            def l_v(t0, t1):
                nc.sync.dma_start(vnv[:, t0:t1, 0:D], vdr[:, t0:t1, :])

            loads = {"l_k": l_k, "l_q": l_q, "l_v": l_v, "m_v": m_v}
            cl = [(0, m_v), (0, lambda: l_k(0, 8)),
                  (1, lambda: l_q(0, 8)), (2, lambda: l_v(0, 8)),
                  (3, lambda: l_k(8, 16)), (4, lambda: l_q(8, 16)),
                  (5, lambda: l_v(8, 16))]
            return loads, cl

        def stage_xforms(h):
            """fp16 casts into zero-padded 128-col blocks + xbar block
            transposes.  Scheduled late enough that the loads (2 heads of
            lead time) are surely complete, so the in-order SP queue never
            blocks on the xbars' inputs."""
            knat, qnat, vnat = nat_by_head[h]
            khp = khpp.tile([128, NKV * 128], F16, tag="khp", name="khp")
            qhp = qhpp.tile([128, NTQ * 128], F16, tag="qhp", name="qhp")
            kt = ktp.tile([128, NKV * 128], F16, tag="kt", name="kt")
            qt = qtp.tile([128, NTQ * 128], F16, tag="qt", name="qt")
            vaug = vaugp.tile([128, NKV * (D + 1)], F16, tag="vaug", name="vaug")
            tiles_by_head[h] = (kt, qt, vaug)

            knv = knat[:].rearrange("p (t e) -> p t e", e=D)
            qnv = qnat[:].rearrange("p (t e) -> p t e", e=D)
            vnv = vnat[:].rearrange("p (t e) -> p t e", e=D + 1)
            khv = khp[:].rearrange("p (t e) -> p t e", e=128)
            qhv = qhp[:].rearrange("p (t e) -> p t e", e=128)
            ktv = kt[:].rearrange("p (t e) -> p t e", e=128)
            qtv = qt[:].rearrange("p (t e) -> p t e", e=128)
            vv = vaug[:].rearrange("p (t e) -> p t e", e=D + 1)
            sh = scale_all[:, h:h + 1]

            def m_kz():
                nc.vector.memset(khv[:, :, D:128], 0.0)

            def m_qz():
                nc.vector.memset(qhv[:, :, D:128], 0.0)

            def c_k(t0, t1):
                nc.vector.tensor_copy(khv[:, t0:t1, 0:D], knv[:, t0:t1, :])

            def c_q(t0, t1):
                nc.vector.tensor_scalar_mul(qhv[:, t0:t1, 0:D],
                                            qnv[:, t0:t1, :], sh)

            def c_v(t0, t1):
                nc.vector.tensor_copy(vv[:, t0:t1, :], vnv[:, t0:t1, :])

            def x_k(t0, t1):
                nc.sync.dma_start(ktv[:, t0:t1, :],
                                  khp[:, t0 * 128:t1 * 128], transpose=True)

            def x_q(t0, t1):
                nc.sync.dma_start(qtv[:, t0:t1, :],
                                  qhp[:, t0 * 128:t1 * 128], transpose=True)

            xf = {"m_kz": m_kz, "m_qz": m_qz, "c_k": c_k, "c_q": c_q,
                  "c_v": c_v, "x_k": x_k, "x_q": x_q}
            cl = []
            if h < 2:
                cl += [(6, m_kz), (7, m_qz)]
            cl += [(12, lambda: c_k(0, 8)), (13, lambda: x_k(0, 8)),
                   (13, lambda: c_q(0, 8)), (14, lambda: x_q(0, 8)),
                   (14, lambda: c_v(0, 8)),
                   (15, lambda: c_k(8, 16)), (16, lambda: x_k(8, 16)),
                   (16, lambda: c_q(8, 16)), (17, lambda: x_q(8, 16)),
                   (17, lambda: c_v(8, 16))]
            return xf, cl

        def stage_head0_prefix():
            """Head 0 prologue, emitted inline.  Ordered so the SP queue
            issues the critical K/Q tile loads first, weaves the xbars in
            as their cast inputs complete, and defers V (the trailing PV
            tolerates ~5 chunks of lag via the pt pool depth)."""
            loads, _ = stage_loads(0)
            l_k, l_q, l_v, m_v = (loads["l_k"], loads["l_q"],
                                  loads["l_v"], loads["m_v"])
            xf, _ = stage_xforms(0)
            m_kz, m_qz = xf["m_kz"], xf["m_qz"]
            c_k, c_q, c_v = xf["c_k"], xf["c_q"], xf["c_v"]
            x_k, x_q = xf["x_k"], xf["x_q"]
            prefix = [
                m_kz, m_qz, m_v,
                lambda: l_k(0, 4), lambda: l_q(0, 4),
                lambda: l_k(4, 8), lambda: l_q(4, 8),
                lambda: l_k(8, 12), lambda: l_v(0, 4),
                lambda: l_k(12, 16), lambda: l_v(4, 8),
                lambda: l_v(8, 12), lambda: l_v(12, 16),
                lambda: c_k(0, 4), lambda: c_q(0, 4),
                lambda: c_k(4, 8), lambda: c_q(4, 8),
                lambda: x_k(0, 4), lambda: x_q(0, 4),
                lambda: x_k(4, 8), lambda: x_q(4, 8),
                lambda: c_k(8, 12), lambda: x_k(8, 12),
                lambda: c_k(12, 16), lambda: x_k(12, 16),
                lambda: c_v(0, 4), lambda: c_v(4, 8),
                lambda: c_v(8, 12), lambda: c_v(12, 16),
            ]
            for f in prefix:
                f()
            rest = [(2, lambda: l_q(8, 16)),
                    (8, lambda: c_q(8, 16)), (9, lambda: x_q(8, 16))]
            return rest

        # ---- flat chunk schedule across all heads ----
        sched = []
        for h in range(nh):
            for qb in range(NQB):
                for ci, (kind, t0, nt) in enumerate(CHUNKS):
                    sched.append((h, qb, kind, t0, nt, ci == CPQB - 1))
        nchunks = len(sched)

        stage_q = []   # (absolute_chunk_pos, closure)
        epi_q = []

        stage_q.extend(stage_head0_prefix())
        if nh > 1:
            _, cl = stage_loads(1)
            stage_q.extend((2 + p, f) for p, f in cl)

        osb_count = [0]

        def emit_qk(i):
            h, qb, kind, t0, nt, _ = sched[i]
            kt, qt, _ = tiles_by_head[h]
            pool = bigp if kind == "B" else smallp
            sc = pool.tile([128, nt * QB], F32, tag="sc", name=f"sc{kind}")
            for j in range(nt):
                t = t0 + j
                nc.tensor.matmul(sc[:, j * QB:(j + 1) * QB],
                                 kt[:, t * 128:(t + 1) * 128],
                                 qt[:, qb * QB:(qb + 1) * QB],
                                 start=True, stop=True)
            return sc

        out_ps_by_qb = {}

        def emit_pv(i):
            h, qb, kind, t0, nt, qb_last = sched[i]
            _, _, vaug = tiles_by_head[h]
            if t0 == 0:
                out_ps_by_qb[(h, qb)] = outp.tile([65, QB], F32, tag="out",
                                                  name="out_ps")
            out_ps = out_ps_by_qb[(h, qb)]
            pt = pt_by_chunk.pop(i)
            for j in range(nt):
                t = t0 + j
                nc.tensor.matmul(out_ps[0:65, :],
                                 vaug[:, t * 65:(t + 1) * 65],
                                 pt[:, j * QB:(j + 1) * QB],
                                 start=(t == 0), stop=(t == NKV - 1))
            if qb_last:
                eps = make_epilogue(h, qb, out_ps)
                epi_q.insert(0, eps[0])   # free the accumulator slot first
                epi_q.extend(eps[1:])

        def make_epilogue(h, qb, out_ps):
            cell = {}

            def c_copy():
                osb = osbp.tile([80, QB], F16, tag="osb", name="osb")
                if osb_count[0] < 2:
                    nc.vector.memset(osb[64:80, :], 0.0)
                osb_count[0] += 1
                nc.vector.tensor_copy(osb[0:65, :], out_ps[0:65, :])
                cell["osb"] = osb

            def c_tr():
                tr = trp.tile([128, 4 * 80], F16, tag="tr", name="tr")
                nc.sync.dma_start(
                    tr[:].rearrange("p (b c) -> p b c", c=80),
                    cell["osb"][:], transpose=True)
                fin = finp.tile([128, 4 * D], F32, tag="fin", name="fin")
                cell["tr"], cell["fin"] = tr, fin

            def c_fin(j):
                tr, fin = cell["tr"], cell["fin"]
                rec = recp.tile([128, 1], F32, tag="rec", name="rec")
                nc.vector.reciprocal(rec[:],
                                     tr[:, j * 80 + D:j * 80 + D + 1])
                nc.vector.tensor_scalar_mul(
                    fin[:, j * D:(j + 1) * D], tr[:, j * 80:j * 80 + D],
                    rec[:])

            odr = o_dram[h].rearrange("(t p) e -> p t e", p=128)

            def fview():
                return cell["fin"][:].rearrange("p (t e) -> p t e", e=D)

            if h == nh - 1 and qb == NQB - 1:
                dmas = [lambda j=j: nc.sync.dma_start(
                    odr[:, qb * 4 + j:qb * 4 + j + 1, :],
                    fview()[:, j:j + 1, :]) for j in range(4)]
            else:
                dmas = [lambda: nc.sync.dma_start(
                    odr[:, qb * 4:(qb + 1) * 4, :], fview())]

            return [c_copy, c_tr] + \
                [lambda j=j: c_fin(j) for j in range(4)] + dmas

        # ---- main pipelined loop over chunks ----
        # per iteration i: QK(i+1) -> EXP(i) -> PV(i-1); the in-order PE
        # then runs QK(i+1) the instant EXP(i-?)'s PSUM read retires,
        # instead of burning that window on PV.
        pt_by_chunk = {}
        sc_cur = emit_qk(0)
        for i in range(nchunks):
            h, qb, kind, t0, nt, qb_last = sched[i]
            # stage the next head (loads + transforms); head 1 waits
            # until head 0's congested prologue window has passed.
            if h + 1 < nh and i == h * CPH + (6 if h == 0 else 0):
                _, cl = stage_loads(h + 1)
                stage_q.extend((i + p, f) for p, f in cl)
                _, cl = stage_xforms(h + 1)
                stage_q.extend((i + p, f) for p, f in cl)
            sc_next = emit_qk(i + 1) if i + 1 < nchunks else None
            pt = ptp.tile([128, nt * QB], F16, tag="pt", name="pt")
            nc.scalar.activation(pt[:], sc_cur[:], EXP, bias=bias_col[:],
                                 scale=1.0)
            pt_by_chunk[i] = pt
            if i > 0:
                emit_pv(i - 1)
            sc_cur = sc_next
            # drain scheduled staging work, then epilogue pieces
            due = [f for p, f in stage_q if p <= i]
            stage_q = [(p, f) for p, f in stage_q if p > i]
            for f in due:
                f()
            ne = 0
            while ne < 2 and epi_q:
                epi_q.pop(0)()
                ne += 1
        emit_pv(nchunks - 1)
        while stage_q:
            stage_q.pop(0)[1]()
        while epi_q:
            epi_q.pop(0)()


_NC_CACHE = {}


def _get_program():
    key = "full"
    if key not in _NC_CACHE:
        _NC_CACHE[key] = build_attention()
    return _NC_CACHE[key]


def kernel(query, key, value, inv_scale_factor):
    """Full-input entry point: shard over 8 cores, run, gather."""
    nc = _get_program()
    q = np.ascontiguousarray(query, dtype=np.float32).reshape(B * H, SQ, D)
    k = np.ascontiguousarray(key, dtype=np.float32).reshape(B * H, SKV, D)
    v = np.ascontiguousarray(value, dtype=np.float32).reshape(B * H, SKV, D)
    inv = np.ascontiguousarray(inv_scale_factor, dtype=np.float32).reshape(B * H)

    hpc = HEADS_PER_CORE
    in_maps = []
    for c in range(N_CORES):
        s = slice(c * hpc, (c + 1) * hpc)
        in_maps.append({
            "query": q[s],
            "key": k[s],
            "value": v[s],
            "inv_scale": inv[s].reshape(1, hpc),
        })
    res = run_bass_kernel_spmd(nc, in_maps, core_ids=list(range(N_CORES)))
    out = np.concatenate([res.results[c]["out"] for c in range(N_CORES)], axis=0)
    return out.reshape(B, H, SQ, D)


# revision 36
# speedup vs baseline: 1.0561x; 1.0543x over previous
"""Trainium2 Bass kernel for batched multi-head attention.

Problem: query/key/value [B=2, H=16, S=2048, D=64] fp32, per-(b,h) divisor
`inv_scale_factor` [B, H, 1, 1].  out = softmax(Q K^T / inv_scale) V.

Sharding: the 32 (b,h) heads are split across 8 NeuronCores, 4 heads per
core, fully data-parallel (no collectives).  Each core runs the same
program on its own 4-head slice.

Per-core algorithm (per head, Sq tiled into q-blocks of 1024):
  - Load Q, K, V naturally ([128 seq, 64 d] tiles), cast to fp16 on DVE.
  - Transpose Q and K tiles on the PE as *regular* fp16 matmuls against an
    fp16 identity (out = tile.T @ I in fp32 PSUM, exact), giving Q^T / K^T
    with d on partitions; the PSUM->SBUF copy casts back to fp16 (exact).
  - scores_T[kv, q] = K^T_tile.T @ Q^T on the PE (fp16 in, fp32 PSUM).
  - P^T = exp(scores_T * (1/inv_scale) - ln 16) on the ACT engine straight
    out of PSUM with fp16 output.  The runtime per-head 1/inv_scale is a
    per-partition scale operand; the -ln 128 bias keeps exp and the
    unnormalized PV accumulator below fp16 max and cancels in the
    normalization.
    No max-subtraction pass is needed.
  - PV uses V augmented with a ones column ([kv, 65] fp16 stationary), so
    the softmax denominator (row 64) falls out of the same accumulating
    matmul chain that contracts P^T with V.
  - The [65, q] fp32 accumulator is copied to SBUF as fp16, transposed
    back on the PE (regular K=128 fp16 matmul against the identity), and
    each [128 q, 64 d] tile is scaled by 1/denominator (DVE reciprocal +
    per-partition tensor_scalar).
"""

import numpy as np

import concourse.bass as bass
import concourse.tile as tile
from concourse import bacc, mybir
from concourse.bass_utils import run_bass_kernel_spmd
from concourse.masks import make_identity

F32 = mybir.dt.float32
F16 = mybir.dt.float16
EXP = mybir.ActivationFunctionType.Exp
LNP = float(np.log(128.0))

B, H, SQ, SKV, D = 2, 16, 2048, 2048, 64
N_CORES = 8
HEADS_PER_CORE = (B * H) // N_CORES  # 4


def build_attention(nh=HEADS_PER_CORE, sq=SQ, skv=SKV, d=D, qblock=1024,
                    num_devices=N_CORES, enable_asserts=False):
    """Build the per-core Bass program. Returns the compiled Bacc module."""
    assert d == 64
    assert sq % 128 == 0 and skv % 128 == 0
    qblock = min(qblock, sq)
    assert sq % qblock == 0
    nchunk = min(512, qblock)          # matmul moving free-dim chunk
    assert qblock % nchunk == 0
    ntq = sq // 128                    # q tiles per head
    nkv = skv // 128                   # kv tiles per head
    nqb = sq // qblock                 # q blocks per head
    ntq_b = qblock // 128              # q tiles per q block

    nc = bacc.Bacc("TRN2", target_bir_lowering=False, debug=False,
                   enable_asserts=enable_asserts, num_devices=num_devices)

    q_dram = nc.dram_tensor("query", [nh, sq, d], F32, kind="ExternalInput").ap()
    k_dram = nc.dram_tensor("key", [nh, skv, d], F32, kind="ExternalInput").ap()
    v_dram = nc.dram_tensor("value", [nh, skv, d], F32, kind="ExternalInput").ap()
    inv_dram = nc.dram_tensor("inv_scale", [1, nh], F32, kind="ExternalInput").ap()
    o_dram = nc.dram_tensor("out", [nh, sq, d], F32, kind="ExternalOutput").ap()

    with tile.TileContext(nc) as tc:
        _attention_body(tc, o_dram, q_dram, k_dram, v_dram, inv_dram,
                        nh, sq, skv, d, qblock, nchunk, ntq, nkv, nqb, ntq_b)

    nc.compile()
    return nc


def _attention_body(tc, o_dram, q_dram, k_dram, v_dram, inv_dram,
                    nh, sq, skv, d, qblock, nchunk, ntq, nkv, nqb, ntq_b):
    nc = tc.nc
    from contextlib import ExitStack
    with ExitStack() as ctx:
        const = ctx.enter_context(tc.tile_pool(name="const", bufs=1))
        qnatp = ctx.enter_context(tc.tile_pool(name="qnat", bufs=2))
        knatp = ctx.enter_context(tc.tile_pool(name="knat", bufs=2))
        vnatp = ctx.enter_context(tc.tile_pool(name="vnat", bufs=2))
        qhp = ctx.enter_context(tc.tile_pool(name="qh", bufs=2))
        khp = ctx.enter_context(tc.tile_pool(name="kh", bufs=2))
        qtp = ctx.enter_context(tc.tile_pool(name="qt", bufs=2))
        ktp = ctx.enter_context(tc.tile_pool(name="kt", bufs=2))
        vaugp = ctx.enter_context(tc.tile_pool(name="vaug", bufs=2))
        ptp = ctx.enter_context(tc.tile_pool(name="pt", bufs=6))
        osbp = ctx.enter_context(tc.tile_pool(name="osb", bufs=2))
        finp = ctx.enter_context(tc.tile_pool(name="fin", bufs=2))
        recp = ctx.enter_context(tc.tile_pool(name="rec", bufs=4))
        scp = ctx.enter_context(tc.tile_pool(name="scps", bufs=2, space="PSUM"))
        outp = ctx.enter_context(tc.tile_pool(name="outps", bufs=1, space="PSUM"))
        tpp = ctx.enter_context(tc.tile_pool(name="tpps", bufs=2, space="PSUM"))

        # --- constants: identities, per-head 1/inv_scale broadcast [128, nh]
        ident = const.tile([128, 128], F32)
        make_identity(nc, ident[:])
        ident_h = const.tile([128, 128], F16)
        nc.vector.tensor_copy(ident_h[:], ident[:])
        inv_sb = const.tile([1, nh], F32)
        nc.sync.dma_start(inv_sb[:], inv_dram[:])
        recip_sb = const.tile([1, nh], F32)
        nc.vector.reciprocal(recip_sb[:], inv_sb[:])
        ones_row = const.tile([1, 128], F32)
        nc.vector.memset(ones_row[:], 1.0)
        bias_col = const.tile([128, 1], F32)
        nc.vector.memset(bias_col[:], -LNP)
        bps = tpp.tile([128, 128], F32, tag="tp")
        nc.tensor.matmul(bps[0:128, 0:nh], ones_row[0:1, 0:128],
                         recip_sb[0:1, 0:nh], start=True, stop=True)
        scale_all = const.tile([128, nh], F32)
        nc.vector.tensor_copy(scale_all[:], bps[0:128, 0:nh])

        def stage_head_loads(h):
            """DMA + fp16 casts for head h; returns tensors + transpose
            closures (one PE transpose + DVE copy each) to be drained
            interleaved with the previous head's main loop."""
            # DMAs and casts split in halves so the first transposes can
            # start as soon as the first half lands (matters for head 0,
            # whose staging is not hidden behind a previous head).
            hq = ntq // 2 * d
            qnat = qnatp.tile([128, ntq * d], F32, tag="qnat", name="qnat")
            qdr = q_dram[h].rearrange("(t p) e -> p t e", p=128)
            qnv = qnat[:].rearrange("p (t e) -> p t e", e=d)
            knat = knatp.tile([128, nkv * d], F32, tag="knat", name="knat")
            kdr = k_dram[h].rearrange("(t p) e -> p t e", p=128)
            knv = knat[:].rearrange("p (t e) -> p t e", e=d)
            vnat = vnatp.tile([128, nkv * (d + 1)], F32, tag="vnat", name="vnat")
            nc.gpsimd.memset(vnat[:], 1.0)
            # queue order: Q half 1, K half 1, V, Q half 2, K half 2 — the
            # first QK + PV need (q-block 0, kt 0, vaug) as early as possible
            nq4 = max(1, ntq // 4)
            nk4 = max(1, nkv // 4)
            nc.sync.dma_start(qnv[:, 0:nq4, :], qdr[:, 0:nq4, :])
            nc.sync.dma_start(knv[:, 0:nk4, :], kdr[:, 0:nk4, :])
            nc.sync.dma_start(qnv[:, nq4:ntq // 2, :], qdr[:, nq4:ntq // 2, :])
            nc.sync.dma_start(knv[:, nk4:nkv // 2, :], kdr[:, nk4:nkv // 2, :])
            # V in quarters: one monolithic dma is 2048 descriptors and
            # occupies a single DMA ring for ~31us, starving the first
            # PVs (which backpressure the EXP stream via the pt pool).
            vnv = vnat[:].rearrange("p (t e) -> p t e", e=d + 1)
            vdr = v_dram[h].rearrange("(t p) e -> p t e", p=128)
            nv4 = max(1, nkv // 4)
            for vq in range(0, nkv, nv4):
                nc.sync.dma_start(vnv[:, vq:vq + nv4, 0:d],
                                  vdr[:, vq:vq + nv4, :])
            nc.sync.dma_start(qnv[:, ntq // 2:, :], qdr[:, ntq // 2:, :])
            nc.sync.dma_start(knv[:, nkv // 2:, :], kdr[:, nkv // 2:, :])
            # the fp16 cast of Q also applies 1/inv_scale, so the exp's scale
            # operand is an immediate (an AP scale costs ~110ns per ACTIVATE)
            sh = scale_all[:, h:h + 1]
            qh16 = qhp.tile([128, ntq * d], F16, tag="qh", name="qh16")
            nc.vector.tensor_scalar_mul(qh16[:, 0:nq4 * d], qnat[:, 0:nq4 * d], sh)
            nc.vector.tensor_scalar_mul(qh16[:, nq4 * d:hq], qnat[:, nq4 * d:hq], sh)
            nc.vector.tensor_scalar_mul(qh16[:, hq:], qnat[:, hq:], sh)
            hk = nkv // 2 * d
            kh16 = khp.tile([128, nkv * d], F16, tag="kh", name="kh16")
            nc.vector.tensor_copy(kh16[:, 0:nk4 * d], knat[:, 0:nk4 * d])
            nc.vector.tensor_copy(kh16[:, nk4 * d:hk], knat[:, nk4 * d:hk])
            nc.vector.tensor_copy(kh16[:, hk:], knat[:, hk:])
            vaug = vaugp.tile([128, nkv * (d + 1)], F16, tag="vaug", name="vaug")
            nc.vector.tensor_copy(vaug[:], vnat[:])

            # Q^T, K^T via regular fp16 matmuls against identity (exact).
            # Rows 64:128 are zero-filled so QK^T can run with a full K=128
            # contraction (zeros contribute nothing): K=64 matmuls keep only
            # half the PE rows active and the clock gate never un-throttles
            # (1.2 GHz); full-row matmuls warm the array to 2.4 GHz.
            # Rows 64:128 only ever hold zeros; pool slots rotate with period
            # 2, so after both slots are zeroed (heads 0 and 1) the reused
            # slots still hold zeros and the memset can be skipped.
            qt = qtp.tile([128, sq], F16, tag="qt", name="qt")
            kt = ktp.tile([128, skv], F16, tag="kt", name="kt")
            if h < 2:
                nc.vector.memset(qt[64:128, :], 0.0)
                nc.vector.memset(kt[64:128, :], 0.0)

            def tq(t):
                psq = tpp.tile([128, 128], F32, tag="tp", name="psq")
                nc.tensor.matmul(psq[0:64, 0:128],
                                 qh16[:, t * d:(t + 1) * d],
                                 ident_h[0:128, 0:128], start=True, stop=True)
                nc.vector.tensor_copy(qt[0:64, t * 128:(t + 1) * 128],
                                      psq[0:64, 0:128])

            def tk(t):
                psk = tpp.tile([128, 128], F32, tag="tp", name="psk")
                nc.tensor.matmul(psk[0:64, 0:128],
                                 kh16[:, t * d:(t + 1) * d],
                                 ident_h[0:128, 0:128], start=True, stop=True)
                nc.vector.tensor_copy(kt[0:64, t * 128:(t + 1) * 128],
                                      psk[0:64, 0:128])

            closures = [lambda t=t: tk(t) for t in range(nkv)]
            closures += [lambda t=t: tq(t) for t in range(ntq)]
            return qt, kt, vaug, closures

        # Head 0: drain only the transposes the first q-block needs (kt 0-2,
        # qt tiles of q-block 0); the rest interleave into its own main loop.
        staged = stage_head_loads(0)
        nk0 = min(6, nkv)
        prefix = staged[3][0:nk0] + staged[3][nkv:nkv + ntq_b]
        rest = staged[3][nk0:nkv] + staged[3][nkv + ntq_b:]
        for f in prefix:
            f()
        staged = staged[:3] + (rest,)

        osb_count = [0]

        def make_epilogue(h, qb, out_ps):
            """Per-q-block epilogue as a list of small closures, drained one
            per kv-iteration so the PE/DVE work hides under ACT's exp.  The
            transpose back to [q, d] is a regular fp16 matmul against the
            identity with a full K=128 contraction (rows 65:128 of osb are
            zeroed once per pool slot) so it doesn't cool the PE clock."""
            cell = {}

            def c_copy():
                osb = osbp.tile([128, qblock], F16, tag="osb", name="osb")
                if osb_count[0] < 2:
                    nc.vector.memset(osb[64:128, :], 0.0)
                osb_count[0] += 1
                nc.vector.tensor_copy(osb[0:65, :], out_ps[0:65, :])
                fin = finp.tile([128, ntq_b * d], F32, tag="fin", name="fin")
                cell["osb"], cell["fin"] = osb, fin

            def c_tile(st):
                pso = tpp.tile([128, 128], F32, tag="tp", name="pso")
                nc.tensor.matmul(pso[0:128, 0:65],
                                 cell["osb"][0:128, st * 128:(st + 1) * 128],
                                 ident_h[0:128, 0:65], start=True, stop=True)
                rec = recp.tile([128, 1], F32, tag="rec", name="rec")
                nc.vector.reciprocal(rec[:], pso[:, 64:65])
                nc.vector.tensor_scalar_mul(
                    cell["fin"][:, st * d:(st + 1) * d], pso[:, 0:d], rec[:])

            def c_dma(p0, p1):
                # store in 2-tile pieces: a single 8-tile dma is 1024
                # descriptors on one ring (~16us) and dominates the tail
                odr = o_dram[h].rearrange("(t p) e -> p t e", p=128)
                fv = cell["fin"][:].rearrange("p (t e) -> p t e", e=d)
                for j in range(p0, p1, 2):
                    nc.sync.dma_start(
                        odr[:, qb * ntq_b + j:qb * ntq_b + j + 2, :],
                        fv[:, j:j + 2, :])

            return [c_copy] + [lambda st=st: c_tile(st) for st in range(ntq_b)] \
                + [lambda: c_dma(0, ntq_b // 2), lambda: c_dma(ntq_b // 2, ntq_b)]

        # ---------------- main loops ----------------
        # Per head, a flat (qb, kv) stream, software-pipelined in emission:
        #   QK(i+1), exp(i), PV(i)
        # so the in-order PE always has the next scores matmul queued while
        # ACT runs exp(i); ACT is the saturated engine.  Background `work`
        # (next head's staging transposes, previous q-block's epilogue) is
        # drained a bit per iteration into the PE/DVE slack so neither
        # q-block nor head boundaries bubble the ACT stream.
        stage_q = []   # next head's staging: MUST be empty before that head
        epi_q = []     # epilogue pieces: only self-dependent, may trail
        niter = nqb * nkv
        for h in range(nh):
            qt, kt, vaug, pending = staged
            stage_q.extend(pending)
            if h + 1 < nh:
                nxt = stage_head_loads(h + 1)
                stage_q.extend(nxt[3])
            else:
                nxt = None

            def emit_qk(it):
                qb, kvt = divmod(it, nkv)
                q0 = qb * qblock
                sc = scp.tile([128, qblock], F32, tag="sc", name="sc")
                for c in range(qblock // nchunk):
                    nc.tensor.matmul(
                        sc[:, c * nchunk:(c + 1) * nchunk],
                        kt[0:128, kvt * 128:(kvt + 1) * 128],
                        qt[0:128, q0 + c * nchunk:q0 + (c + 1) * nchunk],
                        start=True, stop=True)
                return sc

            sc_cur = emit_qk(0)
            out_ps = None
            for it in range(niter):
                qb, kvt = divmod(it, nkv)
                if kvt == 0:
                    out_ps = outp.tile([65, qblock], F32, tag="out",
                                       name="out_ps")
                sc_next = emit_qk(it + 1) if it + 1 < niter else None
                pt = ptp.tile([128, qblock], F16, tag="pt")
                nc.scalar.activation(pt[:], sc_cur[:], EXP,
                                     bias=bias_col[:], scale=1.0)
                for c in range(qblock // nchunk):
                    nc.tensor.matmul(
                        out_ps[0:65, c * nchunk:(c + 1) * nchunk],
                        vaug[:, kvt * (d + 1):(kvt + 1) * (d + 1)],
                        pt[:, c * nchunk:(c + 1) * nchunk],
                        start=(kvt == 0), stop=(kvt == nkv - 1))
                sc_cur = sc_next
                if kvt == nkv - 1:
                    eps = make_epilogue(h, qb, out_ps)
                    epi_q.insert(0, eps[0])  # the PSUM->SBUF copy frees the
                    epi_q.extend(eps[1:])    # accumulator slot: drain first
                budget = 2
                while budget and stage_q and \
                        len(stage_q) > max(0, niter - 2 - it):
                    stage_q.pop(0)()
                    budget -= 1
                if budget and stage_q:
                    stage_q.pop(0)()
                    budget -= 1
                if budget and epi_q:
                    epi_q.pop(0)()
            while stage_q:
                stage_q.pop(0)()
            if nxt is not None:
                staged = nxt[:3] + ([],)

        while epi_q:
            epi_q.pop(0)()


_NC_CACHE = {}


def _get_program():
    key = "full"
    if key not in _NC_CACHE:
        _NC_CACHE[key] = build_attention()
    return _NC_CACHE[key]


def kernel(query, key, value, inv_scale_factor):
    """Full-input entry point: shard over 8 cores, run, gather."""
    nc = _get_program()
    q = np.ascontiguousarray(query, dtype=np.float32).reshape(B * H, SQ, D)
    k = np.ascontiguousarray(key, dtype=np.float32).reshape(B * H, SKV, D)
    v = np.ascontiguousarray(value, dtype=np.float32).reshape(B * H, SKV, D)
    inv = np.ascontiguousarray(inv_scale_factor, dtype=np.float32).reshape(B * H)

    hpc = HEADS_PER_CORE
    in_maps = []
    for c in range(N_CORES):
        s = slice(c * hpc, (c + 1) * hpc)
        in_maps.append({
            "query": q[s],
            "key": k[s],
            "value": v[s],
            "inv_scale": inv[s].reshape(1, hpc),
        })
    res = run_bass_kernel_spmd(nc, in_maps, core_ids=list(range(N_CORES)))
    out = np.concatenate([res.results[c]["out"] for c in range(N_CORES)], axis=0)
    return out.reshape(B, H, SQ, D)



# revision 40
# speedup vs baseline: 1.0696x; 1.0128x over previous
"""Trainium2 Bass kernel for batched multi-head attention.

Problem: query/key/value [B=2, H=16, S=2048, D=64] fp32, per-(b,h) divisor
`inv_scale_factor` [B, H, 1, 1].  out = softmax(Q K^T / inv_scale) V.

Sharding: the 32 (b,h) heads are split across 8 NeuronCores, 4 heads per
core, fully data-parallel (no collectives).  Each core runs the same
program on its own 4-head slice.

Per-core algorithm (per head, Sq tiled into q-blocks of 1024):
  - Load Q, K, V naturally ([128 seq, 64 d] tiles), cast to fp16 on DVE.
  - Transpose Q and K tiles on the PE as *regular* fp16 matmuls against an
    fp16 identity (out = tile.T @ I in fp32 PSUM, exact), giving Q^T / K^T
    with d on partitions; the PSUM->SBUF copy casts back to fp16 (exact).
  - scores_T[kv, q] = K^T_tile.T @ Q^T on the PE (fp16 in, fp32 PSUM).
  - P^T = exp(scores_T * (1/inv_scale) - ln 16) on the ACT engine straight
    out of PSUM with fp16 output.  The runtime per-head 1/inv_scale is a
    per-partition scale operand; the -ln 128 bias keeps exp and the
    unnormalized PV accumulator below fp16 max and cancels in the
    normalization.
    No max-subtraction pass is needed.
  - PV uses V augmented with a ones column ([kv, 65] fp16 stationary), so
    the softmax denominator (row 64) falls out of the same accumulating
    matmul chain that contracts P^T with V.
  - The [65, q] fp32 accumulator is copied to SBUF as fp16, transposed
    back on the PE (regular K=128 fp16 matmul against the identity), and
    each [128 q, 64 d] tile is scaled by 1/denominator (DVE reciprocal +
    per-partition tensor_scalar).
"""

import numpy as np

import concourse.bass as bass
import concourse.tile as tile
from concourse import bacc, mybir
from concourse.bass_utils import run_bass_kernel_spmd
from concourse.masks import make_identity

F32 = mybir.dt.float32
F16 = mybir.dt.float16
EXP = mybir.ActivationFunctionType.Exp
LNP = float(np.log(128.0))

B, H, SQ, SKV, D = 2, 16, 2048, 2048, 64
N_CORES = 8
HEADS_PER_CORE = (B * H) // N_CORES  # 4


def build_attention(nh=HEADS_PER_CORE, sq=SQ, skv=SKV, d=D, qblock=1024,
                    num_devices=N_CORES, enable_asserts=False):
    """Build the per-core Bass program. Returns the compiled Bacc module."""
    assert d == 64
    assert sq % 128 == 0 and skv % 128 == 0
    qblock = min(qblock, sq)
    assert sq % qblock == 0
    nchunk = min(512, qblock)          # matmul moving free-dim chunk
    assert qblock % nchunk == 0
    ntq = sq // 128                    # q tiles per head
    nkv = skv // 128                   # kv tiles per head
    nqb = sq // qblock                 # q blocks per head
    ntq_b = qblock // 128              # q tiles per q block

    nc = bacc.Bacc("TRN2", target_bir_lowering=False, debug=False,
                   enable_asserts=enable_asserts, num_devices=num_devices)

    q_dram = nc.dram_tensor("query", [nh, sq, d], F32, kind="ExternalInput").ap()
    k_dram = nc.dram_tensor("key", [nh, skv, d], F32, kind="ExternalInput").ap()
    v_dram = nc.dram_tensor("value", [nh, skv, d], F32, kind="ExternalInput").ap()
    inv_dram = nc.dram_tensor("inv_scale", [1, nh], F32, kind="ExternalInput").ap()
    o_dram = nc.dram_tensor("out", [nh, sq, d], F32, kind="ExternalOutput").ap()

    with tile.TileContext(nc) as tc:
        _attention_body(tc, o_dram, q_dram, k_dram, v_dram, inv_dram,
                        nh, sq, skv, d, qblock, nchunk, ntq, nkv, nqb, ntq_b)

    nc.compile()
    return nc


def _attention_body(tc, o_dram, q_dram, k_dram, v_dram, inv_dram,
                    nh, sq, skv, d, qblock, nchunk, ntq, nkv, nqb, ntq_b):
    nc = tc.nc
    from contextlib import ExitStack
    with ExitStack() as ctx:
        const = ctx.enter_context(tc.tile_pool(name="const", bufs=1))
        qnatp = ctx.enter_context(tc.tile_pool(name="qnat", bufs=2))
        knatp = ctx.enter_context(tc.tile_pool(name="knat", bufs=2))
        vnatp = ctx.enter_context(tc.tile_pool(name="vnat", bufs=2))
        qhp = ctx.enter_context(tc.tile_pool(name="qh", bufs=2))
        khp = ctx.enter_context(tc.tile_pool(name="kh", bufs=2))
        qtp = ctx.enter_context(tc.tile_pool(name="qt", bufs=2))
        ktp = ctx.enter_context(tc.tile_pool(name="kt", bufs=2))
        vaugp = ctx.enter_context(tc.tile_pool(name="vaug", bufs=2))
        ptp = ctx.enter_context(tc.tile_pool(name="pt", bufs=6))
        osbp = ctx.enter_context(tc.tile_pool(name="osb", bufs=2))
        finp = ctx.enter_context(tc.tile_pool(name="fin", bufs=2))
        recp = ctx.enter_context(tc.tile_pool(name="rec", bufs=4))
        scp = ctx.enter_context(tc.tile_pool(name="scps", bufs=2, space="PSUM"))
        outp = ctx.enter_context(tc.tile_pool(name="outps", bufs=1, space="PSUM"))
        tpp = ctx.enter_context(tc.tile_pool(name="tpps", bufs=2, space="PSUM"))

        # --- constants: identities, per-head 1/inv_scale broadcast [128, nh]
        ident = const.tile([128, 128], F32)
        make_identity(nc, ident[:])
        ident_h = const.tile([128, 128], F16)
        nc.vector.tensor_copy(ident_h[:], ident[:])
        inv_sb = const.tile([1, nh], F32)
        nc.sync.dma_start(inv_sb[:], inv_dram[:])
        recip_sb = const.tile([1, nh], F32)
        nc.vector.reciprocal(recip_sb[:], inv_sb[:])
        ones_row = const.tile([1, 128], F32)
        nc.vector.memset(ones_row[:], 1.0)
        bias_col = const.tile([128, 1], F32)
        nc.vector.memset(bias_col[:], -LNP)
        bps = tpp.tile([128, 128], F32, tag="tp")
        nc.tensor.matmul(bps[0:128, 0:nh], ones_row[0:1, 0:128],
                         recip_sb[0:1, 0:nh], start=True, stop=True)
        scale_all = const.tile([128, nh], F32)
        nc.vector.tensor_copy(scale_all[:], bps[0:128, 0:nh])

        def stage_head_loads(h):
            """DMA + fp16 casts for head h; returns tensors + transpose
            closures (one PE transpose + DVE copy each) to be drained
            interleaved with the previous head's main loop."""
            # DMAs and casts split in halves so the first transposes can
            # start as soon as the first half lands (matters for head 0,
            # whose staging is not hidden behind a previous head).
            hq = ntq // 2 * d
            qnat = qnatp.tile([128, ntq * d], F32, tag="qnat", name="qnat")
            qdr = q_dram[h].rearrange("(t p) e -> p t e", p=128)
            qnv = qnat[:].rearrange("p (t e) -> p t e", e=d)
            knat = knatp.tile([128, nkv * d], F32, tag="knat", name="knat")
            kdr = k_dram[h].rearrange("(t p) e -> p t e", p=128)
            knv = knat[:].rearrange("p (t e) -> p t e", e=d)
            vnat = vnatp.tile([128, nkv * (d + 1)], F32, tag="vnat", name="vnat")
            nc.gpsimd.memset(vnat[:], 1.0)
            # queue order: Q half 1, K half 1, V, Q half 2, K half 2 — the
            # first QK + PV need (q-block 0, kt 0, vaug) as early as possible
            nq4 = max(1, ntq // 4)
            nk4 = max(1, nkv // 4)
            # first halves in pair-sized dmas: the first EXP needs q tiles
            # 0..7 and kt 0..5; pairs land in ~4us/ring instead of ~8us
            # for quarters, pulling the whole ramp in.
            for j in range(0, ntq // 2, 2):
                nc.sync.dma_start(qnv[:, j:j + 2, :], qdr[:, j:j + 2, :])
            for j in range(0, nkv // 2, 2):
                nc.sync.dma_start(knv[:, j:j + 2, :], kdr[:, j:j + 2, :])
            # V in quarters: one monolithic dma is 2048 descriptors and
            # occupies a single DMA ring for ~31us, starving the first
            # PVs (which backpressure the EXP stream via the pt pool).
            vnv = vnat[:].rearrange("p (t e) -> p t e", e=d + 1)
            vdr = v_dram[h].rearrange("(t p) e -> p t e", p=128)
            nv4 = max(1, nkv // 4)
            for vq in range(0, nkv, nv4):
                nc.sync.dma_start(vnv[:, vq:vq + nv4, 0:d],
                                  vdr[:, vq:vq + nv4, :])
            nc.sync.dma_start(qnv[:, ntq // 2:, :], qdr[:, ntq // 2:, :])
            nc.sync.dma_start(knv[:, nkv // 2:, :], kdr[:, nkv // 2:, :])
            # the fp16 cast of Q also applies 1/inv_scale, so the exp's scale
            # operand is an immediate (an AP scale costs ~110ns per ACTIVATE)
            sh = scale_all[:, h:h + 1]
            qh16 = qhp.tile([128, ntq * d], F16, tag="qh", name="qh16")
            nc.vector.tensor_scalar_mul(qh16[:, 0:nq4 * d], qnat[:, 0:nq4 * d], sh)
            nc.vector.tensor_scalar_mul(qh16[:, nq4 * d:hq], qnat[:, nq4 * d:hq], sh)
            nc.vector.tensor_scalar_mul(qh16[:, hq:], qnat[:, hq:], sh)
            hk = nkv // 2 * d
            kh16 = khp.tile([128, nkv * d], F16, tag="kh", name="kh16")
            nc.vector.tensor_copy(kh16[:, 0:nk4 * d], knat[:, 0:nk4 * d])
            nc.vector.tensor_copy(kh16[:, nk4 * d:hk], knat[:, nk4 * d:hk])
            nc.vector.tensor_copy(kh16[:, hk:], knat[:, hk:])
            vaug = vaugp.tile([128, nkv * (d + 1)], F16, tag="vaug", name="vaug")
            nc.vector.tensor_copy(vaug[:], vnat[:])

            # Q^T, K^T via regular fp16 matmuls against identity (exact).
            # Rows 64:128 are zero-filled so QK^T can run with a full K=128
            # contraction (zeros contribute nothing): K=64 matmuls keep only
            # half the PE rows active and the clock gate never un-throttles
            # (1.2 GHz); full-row matmuls warm the array to 2.4 GHz.
            # Rows 64:128 only ever hold zeros; pool slots rotate with period
            # 2, so after both slots are zeroed (heads 0 and 1) the reused
            # slots still hold zeros and the memset can be skipped.
            qt = qtp.tile([128, sq], F16, tag="qt", name="qt")
            kt = ktp.tile([128, skv], F16, tag="kt", name="kt")
            if h < 2:
                nc.vector.memset(qt[64:128, :], 0.0)
                nc.vector.memset(kt[64:128, :], 0.0)

            def tq(t):
                psq = tpp.tile([128, 128], F32, tag="tp", name="psq")
                nc.tensor.matmul(psq[0:64, 0:128],
                                 qh16[:, t * d:(t + 1) * d],
                                 ident_h[0:128, 0:128], start=True, stop=True)
                nc.vector.tensor_copy(qt[0:64, t * 128:(t + 1) * 128],
                                      psq[0:64, 0:128])

            def tk(t):
                psk = tpp.tile([128, 128], F32, tag="tp", name="psk")
                nc.tensor.matmul(psk[0:64, 0:128],
                                 kh16[:, t * d:(t + 1) * d],
                                 ident_h[0:128, 0:128], start=True, stop=True)
                nc.vector.tensor_copy(kt[0:64, t * 128:(t + 1) * 128],
                                      psk[0:64, 0:128])

            closures = [lambda t=t: tk(t) for t in range(nkv)]
            closures += [lambda t=t: tq(t) for t in range(ntq)]
            return qt, kt, vaug, closures

        # Head 0: drain only the transposes the first q-block needs (kt 0-2,
        # qt tiles of q-block 0); the rest interleave into its own main loop.
        staged = stage_head_loads(0)
        nk0 = min(6, nkv)
        prefix = staged[3][0:nk0] + staged[3][nkv:nkv + ntq_b]
        rest = staged[3][nk0:nkv] + staged[3][nkv + ntq_b:]
        for f in prefix:
            f()
        staged = staged[:3] + (rest,)

        osb_count = [0]

        def make_epilogue(h, qb, out_ps):
            """Per-q-block epilogue as a list of small closures, drained one
            per kv-iteration so the PE/DVE work hides under ACT's exp.  The
            transpose back to [q, d] is a regular fp16 matmul against the
            identity with a full K=128 contraction (rows 65:128 of osb are
            zeroed once per pool slot) so it doesn't cool the PE clock."""
            cell = {}

            def c_copy():
                osb = osbp.tile([128, qblock], F16, tag="osb", name="osb")
                if osb_count[0] < 2:
                    nc.vector.memset(osb[64:128, :], 0.0)
                osb_count[0] += 1
                nc.vector.tensor_copy(osb[0:65, :], out_ps[0:65, :])
                fin = finp.tile([128, ntq_b * d], F32, tag="fin", name="fin")
                cell["osb"], cell["fin"] = osb, fin

            def c_tile(st):
                pso = tpp.tile([128, 128], F32, tag="tp", name="pso")
                nc.tensor.matmul(pso[0:128, 0:65],
                                 cell["osb"][0:128, st * 128:(st + 1) * 128],
                                 ident_h[0:128, 0:65], start=True, stop=True)
                rec = recp.tile([128, 1], F32, tag="rec", name="rec")
                nc.vector.reciprocal(rec[:], pso[:, 64:65])
                nc.vector.tensor_scalar_mul(
                    cell["fin"][:, st * d:(st + 1) * d], pso[:, 0:d], rec[:])

            def c_dma(p0, p1):
                # store in 2-tile pieces: a single 8-tile dma is 1024
                # descriptors on one ring (~16us) and dominates the tail
                odr = o_dram[h].rearrange("(t p) e -> p t e", p=128)
                fv = cell["fin"][:].rearrange("p (t e) -> p t e", e=d)
                for j in range(p0, p1, 2):
                    nc.sync.dma_start(
                        odr[:, qb * ntq_b + j:qb * ntq_b + j + 2, :],
                        fv[:, j:j + 2, :])

            return [c_copy] + [lambda st=st: c_tile(st) for st in range(ntq_b)] \
                + [lambda: c_dma(0, ntq_b // 2), lambda: c_dma(ntq_b // 2, ntq_b)]

        # ---------------- main loops ----------------
        # Per head, a flat (qb, kv) stream, software-pipelined in emission:
        #   QK(i+1), exp(i), PV(i)
        # so the in-order PE always has the next scores matmul queued while
        # ACT runs exp(i); ACT is the saturated engine.  Background `work`
        # (next head's staging transposes, previous q-block's epilogue) is
        # drained a bit per iteration into the PE/DVE slack so neither
        # q-block nor head boundaries bubble the ACT stream.
        stage_q = []   # next head's staging: MUST be empty before that head
        epi_q = []     # epilogue pieces: only self-dependent, may trail
        niter = nqb * nkv
        out_ps = None
        for h in range(nh):
            qt, kt, vaug, pending = staged
            stage_q.extend(pending)
            if h + 1 < nh:
                nxt = stage_head_loads(h + 1)
                stage_q.extend(nxt[3])
            else:
                nxt = None

            def emit_qk(it):
                qb, kvt = divmod(it, nkv)
                q0 = qb * qblock
                sc = scp.tile([128, qblock], F32, tag="sc", name="sc")
                for c in range(qblock // nchunk):
                    nc.tensor.matmul(
                        sc[:, c * nchunk:(c + 1) * nchunk],
                        kt[0:128, kvt * 128:(kvt + 1) * 128],
                        qt[0:128, q0 + c * nchunk:q0 + (c + 1) * nchunk],
                        start=True, stop=True)
                return sc

            def emit_pv(pit, ppt):
                # PV for iteration pit, emitted one iteration late so the
                # in-order PE starts the next QK (which gates the next
                # EXP via the double-buffered score slots) the moment an
                # EXP's PSUM read retires, instead of burning that window
                # on PV.  The pt pool depth covers the extra lag.
                nonlocal out_ps
                pqb, pkv = divmod(pit, nkv)
                if pkv == 0:
                    out_ps = outp.tile([65, qblock], F32, tag="out",
                                       name="out_ps")
                for c in range(qblock // nchunk):
                    nc.tensor.matmul(
                        out_ps[0:65, c * nchunk:(c + 1) * nchunk],
                        vaug[:, pkv * (d + 1):(pkv + 1) * (d + 1)],
                        ppt[:, c * nchunk:(c + 1) * nchunk],
                        start=(pkv == 0), stop=(pkv == nkv - 1))
                if pkv == nkv - 1:
                    eps = make_epilogue(h, pqb, out_ps)
                    epi_q.insert(0, eps[0])  # the PSUM->SBUF copy frees the
                    epi_q.extend(eps[1:])    # accumulator slot: drain first

            sc_cur = emit_qk(0)
            prev_pt = None
            for it in range(niter):
                sc_next = emit_qk(it + 1) if it + 1 < niter else None
                pt = ptp.tile([128, qblock], F16, tag="pt")
                nc.scalar.activation(pt[:], sc_cur[:], EXP,
                                     bias=bias_col[:], scale=1.0)
                if prev_pt is not None:
                    emit_pv(it - 1, prev_pt)
                prev_pt = pt
                sc_cur = sc_next
                budget = 2
                while budget and stage_q and \
                        len(stage_q) > max(0, niter - 2 - it):
                    stage_q.pop(0)()
                    budget -= 1
                if budget and stage_q:
                    stage_q.pop(0)()
                    budget -= 1
                if budget and epi_q:
                    epi_q.pop(0)()
            emit_pv(niter - 1, prev_pt)
            while stage_q:
                stage_q.pop(0)()
            if nxt is not None:
                staged = nxt[:3] + ([],)

        while epi_q:
            epi_q.pop(0)()


_NC_CACHE = {}


def _get_program():
    key = "full"
    if key not in _NC_CACHE:
        _NC_CACHE[key] = build_attention()
    return _NC_CACHE[key]


def kernel(query, key, value, inv_scale_factor):
    """Full-input entry point: shard over 8 cores, run, gather."""
    nc = _get_program()
    q = np.ascontiguousarray(query, dtype=np.float32).reshape(B * H, SQ, D)
    k = np.ascontiguousarray(key, dtype=np.float32).reshape(B * H, SKV, D)
    v = np.ascontiguousarray(value, dtype=np.float32).reshape(B * H, SKV, D)
    inv = np.ascontiguousarray(inv_scale_factor, dtype=np.float32).reshape(B * H)

    hpc = HEADS_PER_CORE
    in_maps = []
    for c in range(N_CORES):
        s = slice(c * hpc, (c + 1) * hpc)
        in_maps.append({
            "query": q[s],
            "key": k[s],
            "value": v[s],
            "inv_scale": inv[s].reshape(1, hpc),
        })
    res = run_bass_kernel_spmd(nc, in_maps, core_ids=list(range(N_CORES)))
    out = np.concatenate([res.results[c]["out"] for c in range(N_CORES)], axis=0)
    return out.reshape(B, H, SQ, D)



# revision 41
# speedup vs baseline: 1.0972x; 1.0258x over previous
"""Trainium2 Bass kernel for batched multi-head attention.

Problem: query/key/value [B=2, H=16, S=2048, D=64] fp32, per-(b,h) divisor
`inv_scale_factor` [B, H, 1, 1].  out = softmax(Q K^T / inv_scale) V.

Sharding: the 32 (b,h) heads are split across 8 NeuronCores, 4 heads per
core, fully data-parallel (no collectives).  Each core runs the same
program on its own 4-head slice.

Per-core algorithm (per head, Sq tiled into q-blocks of 1024):
  - Load Q, K, V naturally ([128 seq, 64 d] tiles), cast to fp16 on DVE.
  - Transpose Q and K tiles on the PE as *regular* fp16 matmuls against an
    fp16 identity (out = tile.T @ I in fp32 PSUM, exact), giving Q^T / K^T
    with d on partitions; the PSUM->SBUF copy casts back to fp16 (exact).
  - scores_T[kv, q] = K^T_tile.T @ Q^T on the PE (fp16 in, fp32 PSUM).
  - P^T = exp(scores_T * (1/inv_scale) - ln 16) on the ACT engine straight
    out of PSUM with fp16 output.  The runtime per-head 1/inv_scale is a
    per-partition scale operand; the -ln 128 bias keeps exp and the
    unnormalized PV accumulator below fp16 max and cancels in the
    normalization.
    No max-subtraction pass is needed.
  - PV uses V augmented with a ones column ([kv, 65] fp16 stationary), so
    the softmax denominator (row 64) falls out of the same accumulating
    matmul chain that contracts P^T with V.
  - The [65, q] fp32 accumulator is copied to SBUF as fp16, transposed
    back on the PE (regular K=128 fp16 matmul against the identity), and
    each [128 q, 64 d] tile is scaled by 1/denominator (DVE reciprocal +
    per-partition tensor_scalar).
"""

import numpy as np

import concourse.bass as bass
import concourse.tile as tile
from concourse import bacc, mybir
from concourse.bass_utils import run_bass_kernel_spmd
from concourse.masks import make_identity

F32 = mybir.dt.float32
F16 = mybir.dt.float16
EXP = mybir.ActivationFunctionType.Exp
LNP = float(np.log(128.0))

B, H, SQ, SKV, D = 2, 16, 2048, 2048, 64
N_CORES = 8
HEADS_PER_CORE = (B * H) // N_CORES  # 4


def build_attention(nh=HEADS_PER_CORE, sq=SQ, skv=SKV, d=D, qblock=1024,
                    num_devices=N_CORES, enable_asserts=False):
    """Build the per-core Bass program. Returns the compiled Bacc module."""
    assert d == 64
    assert sq % 128 == 0 and skv % 128 == 0
    qblock = min(qblock, sq)
    assert sq % qblock == 0
    nchunk = min(512, qblock)          # matmul moving free-dim chunk
    assert qblock % nchunk == 0
    ntq = sq // 128                    # q tiles per head
    nkv = skv // 128                   # kv tiles per head
    nqb = sq // qblock                 # q blocks per head
    ntq_b = qblock // 128              # q tiles per q block

    nc = bacc.Bacc("TRN2", target_bir_lowering=False, debug=False,
                   enable_asserts=enable_asserts, num_devices=num_devices)

    q_dram = nc.dram_tensor("query", [nh, sq, d], F32, kind="ExternalInput").ap()
    k_dram = nc.dram_tensor("key", [nh, skv, d], F32, kind="ExternalInput").ap()
    v_dram = nc.dram_tensor("value", [nh, skv, d], F32, kind="ExternalInput").ap()
    inv_dram = nc.dram_tensor("inv_scale", [1, nh], F32, kind="ExternalInput").ap()
    o_dram = nc.dram_tensor("out", [nh, sq, d], F32, kind="ExternalOutput").ap()

    with tile.TileContext(nc) as tc:
        _attention_body(tc, o_dram, q_dram, k_dram, v_dram, inv_dram,
                        nh, sq, skv, d, qblock, nchunk, ntq, nkv, nqb, ntq_b)

    nc.compile()
    return nc


def _attention_body(tc, o_dram, q_dram, k_dram, v_dram, inv_dram,
                    nh, sq, skv, d, qblock, nchunk, ntq, nkv, nqb, ntq_b):
    nc = tc.nc
    from contextlib import ExitStack
    with ExitStack() as ctx:
        const = ctx.enter_context(tc.tile_pool(name="const", bufs=1))
        qnatp = ctx.enter_context(tc.tile_pool(name="qnat", bufs=2))
        knatp = ctx.enter_context(tc.tile_pool(name="knat", bufs=2))
        vnatp = ctx.enter_context(tc.tile_pool(name="vnat", bufs=2))
        qhp = ctx.enter_context(tc.tile_pool(name="qh", bufs=2))
        khp = ctx.enter_context(tc.tile_pool(name="kh", bufs=2))
        qtp = ctx.enter_context(tc.tile_pool(name="qt", bufs=2))
        ktp = ctx.enter_context(tc.tile_pool(name="kt", bufs=2))
        vaugp = ctx.enter_context(tc.tile_pool(name="vaug", bufs=2))
        ptp = ctx.enter_context(tc.tile_pool(name="pt", bufs=6))
        osbp = ctx.enter_context(tc.tile_pool(name="osb", bufs=2))
        finp = ctx.enter_context(tc.tile_pool(name="fin", bufs=2))
        recp = ctx.enter_context(tc.tile_pool(name="rec", bufs=4))
        scp = ctx.enter_context(tc.tile_pool(name="scps", bufs=2, space="PSUM"))
        outp = ctx.enter_context(tc.tile_pool(name="outps", bufs=1, space="PSUM"))
        tpp = ctx.enter_context(tc.tile_pool(name="tpps", bufs=2, space="PSUM"))

        # --- constants: identities, per-head 1/inv_scale broadcast [128, nh]
        ident = const.tile([128, 128], F32)
        make_identity(nc, ident[:])
        ident_h = const.tile([128, 128], F16)
        nc.vector.tensor_copy(ident_h[:], ident[:])
        inv_sb = const.tile([1, nh], F32)
        nc.sync.dma_start(inv_sb[:], inv_dram[:])
        recip_sb = const.tile([1, nh], F32)
        nc.vector.reciprocal(recip_sb[:], inv_sb[:])
        ones_row = const.tile([1, 128], F32)
        nc.vector.memset(ones_row[:], 1.0)
        bias_col = const.tile([128, 1], F32)
        nc.vector.memset(bias_col[:], -LNP)
        bps = tpp.tile([128, 128], F32, tag="tp")
        nc.tensor.matmul(bps[0:128, 0:nh], ones_row[0:1, 0:128],
                         recip_sb[0:1, 0:nh], start=True, stop=True)
        scale_all = const.tile([128, nh], F32)
        nc.vector.tensor_copy(scale_all[:], bps[0:128, 0:nh])

        def stage_head_loads(h):
            """DMA + fp16 casts for head h; returns tensors + transpose
            closures (one PE transpose + DVE copy each) to be drained
            interleaved with the previous head's main loop."""
            # DMAs and casts split in halves so the first transposes can
            # start as soon as the first half lands (matters for head 0,
            # whose staging is not hidden behind a previous head).
            hq = ntq // 2 * d
            qnat = qnatp.tile([128, ntq * d], F32, tag="qnat", name="qnat")
            qdr = q_dram[h].rearrange("(t p) e -> p t e", p=128)
            qnv = qnat[:].rearrange("p (t e) -> p t e", e=d)
            knat = knatp.tile([128, nkv * d], F32, tag="knat", name="knat")
            kdr = k_dram[h].rearrange("(t p) e -> p t e", p=128)
            knv = knat[:].rearrange("p (t e) -> p t e", e=d)
            vnat = vnatp.tile([128, nkv * (d + 1)], F32, tag="vnat", name="vnat")
            nc.gpsimd.memset(vnat[:], 1.0)
            # queue order: Q half 1, K half 1, V, Q half 2, K half 2 — the
            # first QK + PV need (q-block 0, kt 0, vaug) as early as possible
            nq4 = max(1, ntq // 4)
            nk4 = max(1, nkv // 4)
            nc.sync.dma_start(qnv[:, 0:nq4, :], qdr[:, 0:nq4, :])
            nc.sync.dma_start(knv[:, 0:nk4, :], kdr[:, 0:nk4, :])
            nc.sync.dma_start(qnv[:, nq4:ntq // 2, :], qdr[:, nq4:ntq // 2, :])
            nc.sync.dma_start(knv[:, nk4:nkv // 2, :], kdr[:, nk4:nkv // 2, :])
            # V in quarters: one monolithic dma is 2048 descriptors and
            # occupies a single DMA ring for ~31us, starving the first
            # PVs (which backpressure the EXP stream via the pt pool).
            vnv = vnat[:].rearrange("p (t e) -> p t e", e=d + 1)
            vdr = v_dram[h].rearrange("(t p) e -> p t e", p=128)
            nv4 = max(1, nkv // 4)
            for vq in range(0, nkv, nv4):
                nc.sync.dma_start(vnv[:, vq:vq + nv4, 0:d],
                                  vdr[:, vq:vq + nv4, :])
            nc.sync.dma_start(qnv[:, ntq // 2:, :], qdr[:, ntq // 2:, :])
            nc.sync.dma_start(knv[:, nkv // 2:, :], kdr[:, nkv // 2:, :])
            # the fp16 cast of Q also applies 1/inv_scale, so the exp's scale
            # operand is an immediate (an AP scale costs ~110ns per ACTIVATE)
            sh = scale_all[:, h:h + 1]
            qh16 = qhp.tile([128, ntq * d], F16, tag="qh", name="qh16")
            nc.vector.tensor_scalar_mul(qh16[:, 0:nq4 * d], qnat[:, 0:nq4 * d], sh)
            nc.vector.tensor_scalar_mul(qh16[:, nq4 * d:hq], qnat[:, nq4 * d:hq], sh)
            nc.vector.tensor_scalar_mul(qh16[:, hq:], qnat[:, hq:], sh)
            hk = nkv // 2 * d
            kh16 = khp.tile([128, nkv * d], F16, tag="kh", name="kh16")
            nc.vector.tensor_copy(kh16[:, 0:nk4 * d], knat[:, 0:nk4 * d])
            nc.vector.tensor_copy(kh16[:, nk4 * d:hk], knat[:, nk4 * d:hk])
            nc.vector.tensor_copy(kh16[:, hk:], knat[:, hk:])
            vaug = vaugp.tile([128, nkv * (d + 1)], F16, tag="vaug", name="vaug")
            nc.vector.tensor_copy(vaug[:], vnat[:])

            # Q^T, K^T via regular fp16 matmuls against identity (exact).
            # Rows 64:128 are zero-filled so QK^T can run with a full K=128
            # contraction (zeros contribute nothing): K=64 matmuls keep only
            # half the PE rows active and the clock gate never un-throttles
            # (1.2 GHz); full-row matmuls warm the array to 2.4 GHz.
            # Rows 64:128 only ever hold zeros; pool slots rotate with period
            # 2, so after both slots are zeroed (heads 0 and 1) the reused
            # slots still hold zeros and the memset can be skipped.
            qt = qtp.tile([128, sq], F16, tag="qt", name="qt")
            kt = ktp.tile([128, skv], F16, tag="kt", name="kt")
            if h < 2:
                nc.vector.memset(qt[64:128, :], 0.0)
                nc.vector.memset(kt[64:128, :], 0.0)

            def tq(t):
                psq = tpp.tile([128, 128], F32, tag="tp", name="psq")
                nc.tensor.matmul(psq[0:64, 0:128],
                                 qh16[:, t * d:(t + 1) * d],
                                 ident_h[0:128, 0:128], start=True, stop=True)
                nc.vector.tensor_copy(qt[0:64, t * 128:(t + 1) * 128],
                                      psq[0:64, 0:128])

            def tk(t):
                psk = tpp.tile([128, 128], F32, tag="tp", name="psk")
                nc.tensor.matmul(psk[0:64, 0:128],
                                 kh16[:, t * d:(t + 1) * d],
                                 ident_h[0:128, 0:128], start=True, stop=True)
                nc.vector.tensor_copy(kt[0:64, t * 128:(t + 1) * 128],
                                      psk[0:64, 0:128])

            closures = [lambda t=t: tk(t) for t in range(nkv)]
            closures += [lambda t=t: tq(t) for t in range(ntq)]
            return qt, kt, vaug, closures

        # Head 0: drain only the transposes the first q-block needs (kt 0-2,
        # qt tiles of q-block 0); the rest interleave into its own main loop.
        staged = stage_head_loads(0)
        nk0 = min(6, nkv)
        prefix = staged[3][0:nk0] + staged[3][nkv:nkv + ntq_b]
        rest = staged[3][nk0:nkv] + staged[3][nkv + ntq_b:]
        for f in prefix:
            f()
        staged = staged[:3] + (rest,)

        osb_count = [0]

        def make_epilogue(h, qb, out_ps):
            """Per-q-block epilogue as a list of small closures, drained one
            per kv-iteration so the PE/DVE work hides under ACT's exp.  The
            transpose back to [q, d] is a regular fp16 matmul against the
            identity with a full K=128 contraction (rows 65:128 of osb are
            zeroed once per pool slot) so it doesn't cool the PE clock."""
            cell = {}

            def c_copy():
                osb = osbp.tile([128, qblock], F16, tag="osb", name="osb")
                if osb_count[0] < 2:
                    nc.vector.memset(osb[64:128, :], 0.0)
                osb_count[0] += 1
                nc.vector.tensor_copy(osb[0:65, :], out_ps[0:65, :])
                fin = finp.tile([128, ntq_b * d], F32, tag="fin", name="fin")
                cell["osb"], cell["fin"] = osb, fin

            def c_tile(st):
                pso = tpp.tile([128, 128], F32, tag="tp", name="pso")
                nc.tensor.matmul(pso[0:128, 0:65],
                                 cell["osb"][0:128, st * 128:(st + 1) * 128],
                                 ident_h[0:128, 0:65], start=True, stop=True)
                rec = recp.tile([128, 1], F32, tag="rec", name="rec")
                nc.vector.reciprocal(rec[:], pso[:, 64:65])
                nc.vector.tensor_scalar_mul(
                    cell["fin"][:, st * d:(st + 1) * d], pso[:, 0:d], rec[:])

            def c_dma(p0, p1):
                # store in 2-tile pieces: a single 8-tile dma is 1024
                # descriptors on one ring (~16us) and dominates the tail
                odr = o_dram[h].rearrange("(t p) e -> p t e", p=128)
                fv = cell["fin"][:].rearrange("p (t e) -> p t e", e=d)
                for j in range(p0, p1, 2):
                    nc.sync.dma_start(
                        odr[:, qb * ntq_b + j:qb * ntq_b + j + 2, :],
                        fv[:, j:j + 2, :])

            return [c_copy] + [lambda st=st: c_tile(st) for st in range(ntq_b)] \
                + [lambda: c_dma(0, ntq_b // 2), lambda: c_dma(ntq_b // 2, ntq_b)]

        # ---------------- main loops ----------------
        # Per head, a flat (qb, kv) stream, software-pipelined in emission:
        #   QK(i+1), exp(i), PV(i)
        # so the in-order PE always has the next scores matmul queued while
        # ACT runs exp(i); ACT is the saturated engine.  Background `work`
        # (next head's staging transposes, previous q-block's epilogue) is
        # drained a bit per iteration into the PE/DVE slack so neither
        # q-block nor head boundaries bubble the ACT stream.
        stage_q = []   # next head's staging: MUST be empty before that head
        epi_q = []     # epilogue pieces: only self-dependent, may trail
        niter = nqb * nkv
        out_ps = None
        for h in range(nh):
            qt, kt, vaug, pending = staged
            stage_q.extend(pending)
            if h + 1 < nh:
                nxt = stage_head_loads(h + 1)
                stage_q.extend(nxt[3])
            else:
                nxt = None

            def emit_qk(it):
                qb, kvt = divmod(it, nkv)
                q0 = qb * qblock
                sc = scp.tile([128, qblock], F32, tag="sc", name="sc")
                for c in range(qblock // nchunk):
                    nc.tensor.matmul(
                        sc[:, c * nchunk:(c + 1) * nchunk],
                        kt[0:128, kvt * 128:(kvt + 1) * 128],
                        qt[0:128, q0 + c * nchunk:q0 + (c + 1) * nchunk],
                        start=True, stop=True)
                return sc

            def emit_pv(pit, ppt):
                # PV for iteration pit, emitted one iteration late so the
                # in-order PE starts the next QK (which gates the next
                # EXP via the double-buffered score slots) the moment an
                # EXP's PSUM read retires, instead of burning that window
                # on PV.  The pt pool depth covers the extra lag.
                nonlocal out_ps
                pqb, pkv = divmod(pit, nkv)
                if pkv == 0:
                    out_ps = outp.tile([65, qblock], F32, tag="out",
                                       name="out_ps")
                for c in range(qblock // nchunk):
                    nc.tensor.matmul(
                        out_ps[0:65, c * nchunk:(c + 1) * nchunk],
                        vaug[:, pkv * (d + 1):(pkv + 1) * (d + 1)],
                        ppt[:, c * nchunk:(c + 1) * nchunk],
                        start=(pkv == 0), stop=(pkv == nkv - 1))
                if pkv == nkv - 1:
                    eps = make_epilogue(h, pqb, out_ps)
                    epi_q.insert(0, eps[0])  # the PSUM->SBUF copy frees the
                    epi_q.extend(eps[1:])    # accumulator slot: drain first

            sc_cur = emit_qk(0)
            prev_pt = None
            for it in range(niter):
                sc_next = emit_qk(it + 1) if it + 1 < niter else None
                pt = ptp.tile([128, qblock], F16, tag="pt")
                nc.scalar.activation(pt[:], sc_cur[:], EXP,
                                     bias=bias_col[:], scale=1.0)
                if prev_pt is not None:
                    emit_pv(it - 1, prev_pt)
                prev_pt = pt
                sc_cur = sc_next
                budget = 2
                while budget and stage_q and \
                        len(stage_q) > max(0, niter - 2 - it):
                    stage_q.pop(0)()
                    budget -= 1
                if budget and stage_q:
                    stage_q.pop(0)()
                    budget -= 1
                if budget and epi_q:
                    epi_q.pop(0)()
            emit_pv(niter - 1, prev_pt)
            while stage_q:
                stage_q.pop(0)()
            if nxt is not None:
                staged = nxt[:3] + ([],)

        while epi_q:
            epi_q.pop(0)()


_NC_CACHE = {}


def _get_program():
    key = "full"
    if key not in _NC_CACHE:
        _NC_CACHE[key] = build_attention()
    return _NC_CACHE[key]


def kernel(query, key, value, inv_scale_factor):
    """Full-input entry point: shard over 8 cores, run, gather."""
    nc = _get_program()
    q = np.ascontiguousarray(query, dtype=np.float32).reshape(B * H, SQ, D)
    k = np.ascontiguousarray(key, dtype=np.float32).reshape(B * H, SKV, D)
    v = np.ascontiguousarray(value, dtype=np.float32).reshape(B * H, SKV, D)
    inv = np.ascontiguousarray(inv_scale_factor, dtype=np.float32).reshape(B * H)

    hpc = HEADS_PER_CORE
    in_maps = []
    for c in range(N_CORES):
        s = slice(c * hpc, (c + 1) * hpc)
        in_maps.append({
            "query": q[s],
            "key": k[s],
            "value": v[s],
            "inv_scale": inv[s].reshape(1, hpc),
        })
    res = run_bass_kernel_spmd(nc, in_maps, core_ids=list(range(N_CORES)))
    out = np.concatenate([res.results[c]["out"] for c in range(N_CORES)], axis=0)
    return out.reshape(B, H, SQ, D)

